# revision 22
# baseline (speedup 1.0000x reference)
"""Trainium2 Bass kernel for nn_Polynomial: out = poly_basis(x) @ W.T + bias.

x: [500000, 8] f32.  basis = all 164 monomials of total degree 1..3 over the
8 features.  weight: [64, 164], bias: [64].

Strategy (pure data parallel over 8 cores, 62500 rows each, padded to 64512):
  - rows-on-partitions, fp16 compute path (164-term basis values are < 150,
    fp16's 11-bit mantissa keeps the end-to-end error ~1e-3, far inside the
    2e-2 gate).
  - basis built COLUMN-major in SBUF (b3T [128, 168, g]) so every DVE
    operand of the product ops is packed 2-byte -> DVE 2x mode; the
    broadcast multiplier x_k is a [128, 1, g] stride-0-middle AP whose last
    dim is packed.
  - per 128-row group: one [128,128] fp16 PE transpose (chunk A: x, pairs,
    triples k<=6) writing fp16 straight into PSUM; per triad of 3 groups one
    packed [128, 37, 3] transpose (chunk B: k=7 triples + const/bias col).
  - fp16 matmuls: 3x (ap=64) against wa16 plus one block-diagonal (ap=192)
    against wbd16 accumulate into a [128, 3, 64] f32 PSUM tile.
  - evacuations split across DVE (2x fp16) and ACT; pairs products + const
    memset on the otherwise idle Pool(gpsimd) engine.
  - weights are pre-permuted AND pre-cast to fp16 on the host (wa16, wbd16),
    bias rides as basis column 164 with weight row = bias.
"""

import numpy as np

import concourse.bass as bass
import concourse.bacc as bacc
import concourse.mybir as mybir
from concourse import bass_utils
from concourse import tile
from concourse.masks import make_identity

IN_F = 8
OUT_F = 64
K_TOT = 165  # 164 monomials + 1 const column (fused bias)
KA = 128     # chunk-a columns (one 128x128 transpose per group)
KB = K_TOT - KA  # 37
BASIS_COLS = 168  # padded col count of the col-major basis tile

G = 63
ROWS_PER_SUPER = 128 * G  # 8064
N_CORES = 8
N_ROWS = 500000
ROWS_PER_CORE_RAW = N_ROWS // N_CORES  # 62500
N_SUPER = -(-ROWS_PER_CORE_RAW // ROWS_PER_SUPER)  # 8
ROWS_PER_CORE = N_SUPER * ROWS_PER_SUPER  # 64512

F32 = mybir.dt.float32
F16 = mybir.dt.float16


def _pair_off(j: int) -> int:
    return j * (j + 1) // 2


def _trip_off(k: int) -> int:
    return k * (k + 1) * (k + 2) // 6


# Basis column layout (165 live columns):
#   [0..8)    x_i
#   [8..44)   x_i * x_j      (i<=j), col = 8 + _pair_off(j) + i
#   [44..164) x_i x_j x_k    (i<=j<=k), col = 44 + _trip_off(k) + _pair_off(j) + i
#   [164]     1.0 (bias column)


def _term_col(e) -> int:
    facs = []
    for f in range(IN_F):
        facs += [f] * int(e[f])
    if len(facs) == 1:
        return facs[0]
    if len(facs) == 2:
        i, j = facs
        return 8 + _pair_off(j) + i
    i, j, k = facs
    return 44 + _trip_off(k) + _pair_off(j) + i


def _exponents() -> np.ndarray:
    deg = np.arange(4)
    comb = np.array(np.meshgrid(*([deg] * IN_F))).T.reshape(-1, IN_F)
    s = comb.sum(axis=1)
    nz = (comb != 0).sum(axis=1)
    keep = ((nz == 1) & (s <= 3)) | ((nz > 1) & (s <= 3))
    return comb[keep].astype(np.int32)


def make_wtilde(weight: np.ndarray, bias: np.ndarray) -> np.ndarray:
    """Permute reference weight [64, 164] into W~ [165, 64] matching the
    on-chip basis column order; row 164 is the bias."""
    E = _exponents()
    wt = np.zeros((K_TOT, OUT_F), np.float32)
    for t in range(E.shape[0]):
        wt[_term_col(E[t])] += weight[:, t].astype(np.float32)
    wt[K_TOT - 1] = bias.astype(np.float32)
    return wt


def make_weights(weight: np.ndarray, bias: np.ndarray):
    """Host-side fp16 weight prep: wa16 [128, 64] for chunk A; wbd16
    [111, 192] block-diagonal for the packed 3-group chunk-B matmul.

    The packed chunk-B transpose input is b3T[:, 128:165, q0:q0+3] with free
    dims (c: 37, qi: 3) flattened c-major, so transposed-out partition
    r = 3*c + qi and wbd16[r, 64*qi + o] = wt[128 + c, o].
    """
    wt = make_wtilde(weight, bias)
    wa16 = wt[0:KA].astype(np.float16)
    wbd16 = np.zeros((KB * 3, 3 * OUT_F), np.float16)
    for c in range(KB):
        for qi in range(3):
            wbd16[3 * c + qi, 64 * qi : 64 * qi + 64] = wt[KA + c].astype(np.float16)
    return {"wa16": np.ascontiguousarray(wa16), "wbd16": np.ascontiguousarray(wbd16)}


def poly_tile_kernel(tc, x_ap, wa_ap, wbd_ap, out_ap, g: int = G, bench_reps=None):
    """x_ap: [rows, 8] f32, wa_ap: [128, 64] f16, wbd_ap: [111, 192] f16,
    out_ap: [rows, 64] f32; rows must be a multiple of 128*g; g % 3 == 0."""
    nc = tc.nc
    rows = x_ap.shape[0]
    assert rows % (128 * g) == 0 and g % 3 == 0
    n_super = rows // (128 * g)

    from contextlib import ExitStack

    with ExitStack() as ctx:
        cpool = ctx.enter_context(tc.tile_pool(name="cpool", bufs=1))
        xpool = ctx.enter_context(tc.tile_pool(name="xpool", bufs=3))
        bpool = ctx.enter_context(tc.tile_pool(name="bpool", bufs=2))
        tpool = ctx.enter_context(tc.tile_pool(name="tpool", bufs=6))
        opool = ctx.enter_context(tc.tile_pool(name="opool", bufs=3))
        pst = ctx.enter_context(tc.tile_pool(name="pst", bufs=3, space="PSUM"))
        psb = ctx.enter_context(tc.tile_pool(name="psb", bufs=2, space="PSUM"))
        pso = ctx.enter_context(tc.tile_pool(name="pso", bufs=3, space="PSUM"))

        ident = cpool.tile([128, 128], F16)
        make_identity(nc, ident[:])
        wa = cpool.tile([KA, OUT_F], F16)
        wbd = cpool.tile([KB * 3, 3 * OUT_F], F16)
        nc.sync.dma_start(out=wa[:], in_=wa_ap)
        nc.sync.dma_start(out=wbd[:], in_=wbd_ap)

        xv = x_ap.rearrange("(t p g) f -> t p g f", p=128, g=g)
        ov = out_ap.rearrange("(t p g) f -> t p g f", p=128, g=g)

        def do_supertile(t):
            x3 = xpool.tile([128, g, IN_F], F32, tag="x3")
            nc.scalar.dma_start(out=x3[:], in_=xv[t])

            # col-major chunk-A basis: b3[:, c, gi] = basis column c (0..128)
            b3 = bpool.tile([128, KA, g], F16, tag="b3")
            # chunk B: [tri][c][qi] layout so a triad slice [128, 111] is
            # contiguous for the packed transpose; c-major partition order
            # r = 3*c + qi matches wbd16.
            bB = bpool.tile([128, g // 3, KB * 3], F16, tag="bB")
            bBv = bB[:].rearrange("p t (c q) -> p t c q", c=KB, q=3)
            # cast x f32 -> fp16 into cols 0..8 (Pool, per-partition strided)
            nc.gpsimd.tensor_copy(out=b3[:, 0:IN_F, :], in_=x3[:].rearrange("p g f -> p f g"))
            # const/bias column (chunk-B col 36)
            nc.gpsimd.memset(bBv[:, :, KB - 1 : KB, :], 1.0)
            # pairs: col 8+po(j)+i = x_i * x_j; small ones on Pool(gpsimd),
            # wide ones on DVE (2x fp16)
            for j in range(IN_F):
                w_ = j + 1
                o = 8 + _pair_off(j)
                eng = nc.gpsimd if j < 6 else nc.vector
                eng.tensor_mul(
                    out=b3[:, o : o + w_, :],
                    in0=b3[:, 0:w_, :],
                    in1=b3[:, j : j + 1, :].broadcast_to([128, w_, g]),
                )
            # triples k<=6 on DVE (2x fp16): col 44+to(k)+po(j)+i
            for k in range(IN_F - 1):
                w_ = _pair_off(k + 1)
                o = 44 + _trip_off(k)
                nc.vector.tensor_mul(
                    out=b3[:, o : o + w_, :],
                    in0=b3[:, 8 : 8 + w_, :],
                    in1=b3[:, k : k + 1, :].broadcast_to([128, w_, g]),
                )
            # triples k=7 (36 cols) into chunk B [tri, c, qi] (DVE, packed)
            pairs4 = b3[:, 8:44, :].rearrange("p c (t q) -> p t c q", q=3)
            x74 = b3[:, 7:8, :].rearrange("p c (t q) -> p t c q", q=3)
            nc.vector.tensor_mul(
                out=bBv[:, :, 0 : KB - 1, :],
                in0=pairs4,
                in1=x74.broadcast_to([128, g // 3, KB - 1, 3]),
            )

            out3 = opool.tile([128, g, OUT_F], F32, tag="out3")
            for s0 in range(0, g, 6):
                nq = min(6, g - s0)  # groups in this sextet (6, or 3 tail)
                ntri = nq // 3
                # chunk-A transposes: fp16 PE transpose -> fp16 PSUM
                psA6 = pst.tile([128, 6, 128], F16, tag="psA6")
                for qi in range(nq):
                    nc.tensor.transpose(
                        psA6[:, qi, :], b3[:, 0:KA, s0 + qi], ident[:]
                    )
                # packed chunk-B transposes: [128, 111] -> [111, 128]
                psB6 = psb.tile([KB * 3, 2, 128], F16, tag="psB6")
                for ti in range(ntri):
                    nc.tensor.transpose(psB6[:, ti, :], bB[:, s0 // 3 + ti, :], ident[:])

                sbA6 = tpool.tile([128, 6, 128], F16, tag="sbA6")
                sbB6 = tpool.tile([KB * 3, 2, 128], F16, tag="sbB6")
                even = (s0 // 6) % 2 == 0
                eva = nc.vector.tensor_copy if even else nc.scalar.copy
                eva(out=sbA6[:, 0:nq, :], in_=psA6[:, 0:nq, :])
                nc.scalar.copy(out=sbB6[:, 0:ntri, :], in_=psB6[:, 0:ntri, :])

                for ti in range(ntri):
                    q0 = s0 + 3 * ti
                    po3 = pso.tile([128, 3, OUT_F], F32, tag="po3")
                    nc.tensor.matmul(po3[:], lhsT=sbB6[:, ti, :], rhs=wbd[:],
                                     start=True, stop=False, skip_group_check=True)
                    for qi in range(3):
                        nc.tensor.matmul(po3[:, qi, :],
                                         lhsT=sbA6[:, 3 * ti + qi, :], rhs=wa[:],
                                         start=False, stop=(qi == 2),
                                         skip_group_check=True)
                    # out evac: opposite engine from the sbA6 evac this sextet
                    if even:
                        nc.scalar.copy(out=out3[:, q0 : q0 + 3, :], in_=po3[:])
                    else:
                        nc.vector.tensor_copy(out=out3[:, q0 : q0 + 3, :], in_=po3[:])

            # split the output DMA across both HWDGE queues (SP + ACT)
            gs = 36
            nc.sync.dma_start(out=ov[t][:, 0:gs, :], in_=out3[:, 0:gs, :])
            nc.scalar.dma_start(out=ov[t][:, gs:g, :], in_=out3[:, gs:g, :])

        if bench_reps is None:
            for t in range(n_super):
                do_supertile(t)
        else:
            with tc.For_i(0, bench_reps, 1):
                do_supertile(0)


_CACHED_NC = {}


def build_nc(rows_per_core: int = ROWS_PER_CORE, g: int = G, bench_reps=None):
    key = (rows_per_core, g, bench_reps)
    if key not in _CACHED_NC:
        nc = bacc.Bacc("TRN2", target_bir_lowering=False, debug=False, num_devices=N_CORES)
        x_d = nc.dram_tensor("x", [rows_per_core, IN_F], F32, kind="ExternalInput")
        wa_d = nc.dram_tensor("wa16", [KA, OUT_F], F16, kind="ExternalInput")
        wbd_d = nc.dram_tensor("wbd16", [KB * 3, 3 * OUT_F], F16, kind="ExternalInput")
        o_d = nc.dram_tensor("out", [rows_per_core, OUT_F], F32, kind="ExternalOutput")
        with tile.TileContext(nc) as tc:
            poly_tile_kernel(tc, x_d.ap(), wa_d.ap(), wbd_d.ap(), o_d.ap(), g=g,
                             bench_reps=bench_reps)
        nc.compile()
        _CACHED_NC[key] = nc
    return _CACHED_NC[key]


def make_inmaps(x: np.ndarray, weight: np.ndarray, bias: np.ndarray,
                rows_per_core: int = ROWS_PER_CORE):
    """Shard x row-wise over the 8 cores (zero-padded); replicate weights."""
    w = make_weights(np.asarray(weight, np.float32), np.asarray(bias, np.float32))
    in_maps = []
    for c in range(N_CORES):
        shard = x[c * ROWS_PER_CORE_RAW : (c + 1) * ROWS_PER_CORE_RAW]
        xpad = np.zeros((rows_per_core, IN_F), np.float32)
        xpad[: min(shard.shape[0], rows_per_core)] = shard[:rows_per_core]
        in_maps.append({"x": xpad, **w})
    return in_maps


def kernel(x, weight, bias):
    x = np.ascontiguousarray(np.asarray(x, dtype=np.float32))
    nc = build_nc()
    in_maps = make_inmaps(x, weight, bias)
    res = bass_utils.run_bass_kernel_spmd(nc, in_maps, core_ids=list(range(N_CORES)))
    outs = [r["out"][:ROWS_PER_CORE_RAW] for r in res.results]
    return np.concatenate(outs, axis=0)
